# revision 7
# baseline (speedup 1.0000x reference)
"""CapsuleLayer dynamic-routing kernel for 8 Trainium2 NeuronCores.

I-sharding: each core owns 144 of the 1152 input capsules.

Create phase (all bf16 on the PE):
  - s0 = sum_i hat_i via 9 stacked matmuls: lhsT = xs[(i16,k8)=128, b],
    rhs = ws[(i16,k8)=128, nd] accumulated in one PSUM chain.
  - hat_i = X_i^T W_i per i: 144 serial bf16 matmuls [8,128]x[8,512],
    PSUM drained to SBUF hat (bf16) by copies alternating DVE/ScalarE.
  - AllReduce(s0) overlaps the hat matmul/copy stream.

Routing (b on partitions, free-dim elementwise on DVE at 2x bf16):
  - reduces done as binary add-trees (TT 2x) instead of 1x tensor_reduce.
  - c-broadcast over d materialized by ScalarE (cexp) in half-blocks so
    the DVE multiply gets contiguous bf16 operands.
  - optional gpsimd side-lane takes the last GP_I capsules per pass.
  - Cross-core: AllReduce of s_partial [128,512] fp32, 3x.
Every core computes the identical final output; core 0's is returned.
"""

import os
import numpy as np
import ml_dtypes

import concourse.bass as bass
import concourse.bacc as bacc
import concourse.tile as tile
import concourse.mybir as mybir
from concourse import bass_utils

B, I, DIN = 128, 1152, 8
N, D = 32, 16
ND = N * D  # 512
NCORES = 8
IL = I // NCORES  # 144
EPS = 1e-7
ROUTINGS = 3
F32 = mybir.dt.float32
BF16 = mybir.dt.bfloat16

CH = 3        # i-chunk for X/W streaming in create (48 chunks)
BI = 8        # i-block for DVE routing passes
GP_I = 24     # trailing i's handled by gpsimd (multiple of 4; 0 disables)
BIG = 4       # gpsimd i-block
NCHUNK = 9    # stacked s0 chunks (16 i's each)

BF = np.dtype(ml_dtypes.bfloat16)


def _ap(ap: bass.AP, dims) -> bass.AP:
    """Rebuild `ap` with explicit free [step,count] dims (partition dim kept)."""
    return bass.AP(tensor=ap.tensor, offset=ap.offset, ap=[ap.ap[0]] + list(dims))


def _off(ap: bass.AP, elems: int, dims) -> bass.AP:
    """Like _ap but with an extra offset in elements of the tensor dtype."""
    return bass.AP(
        tensor=ap.tensor,
        offset=ap.offset + elems,
        ap=[ap.ap[0]] + list(dims),
    )


def build_nc():
    nc = bacc.Bacc(
        "TRN2",
        target_bir_lowering=False,
        debug=False,
        enable_asserts=True,
        num_devices=NCORES,
    )
    x_d = nc.dram_tensor("x", [DIN, IL, B], BF16, kind="ExternalInput").ap()
    w_d = nc.dram_tensor("w", [DIN, IL, ND], BF16, kind="ExternalInput").ap()
    xs_d = nc.dram_tensor("xs", [128, NCHUNK, B], BF16, kind="ExternalInput").ap()
    ws_d = nc.dram_tensor("ws", [128, NCHUNK, ND], BF16, kind="ExternalInput").ap()
    out_d = nc.dram_tensor("out", [B, ND], F32, kind="ExternalOutput").ap()

    DVE_I = IL - GP_I          # i's handled by DVE blocks
    NBLK = DVE_I // BI
    NGBLK = GP_I // BIG

    with tile.TileContext(nc) as tc:
        with (
            tc.tile_pool(name="big", bufs=1) as big,
            tc.tile_pool(name="stream", bufs=2) as stream,
            tc.tile_pool(name="cexpp", bufs=2) as cexpp,
            tc.tile_pool(name="ps", bufs=2, space="PSUM") as pspool,
            tc.tile_pool(name="ps0", bufs=1, space="PSUM") as ps0pool,
            tc.tile_pool(name="dram", bufs=1, space="DRAM") as dram,
        ):
            hat = big.tile([B, IL * ND], BF16)       # 147.5 KB/part
            bb = big.tile([B, IL * N], BF16)         # 9.2 KB
            ee = big.tile([B, IL * N], BF16)         # 9.2 KB
            s_acc = big.tile([B, ND], F32)
            s_accg = big.tile([B, ND], F32)
            outv = big.tile([B, ND], F32)
            outbf = big.tile([B, ND], BF16)
            eps_t = big.tile([B, 1], F32)
            tmp = big.tile([B, 12 * ND], BF16)       # 12 KB DVE scratch
            tmpg = big.tile([B, 3584], BF16)         # 7 KB gpsimd scratch
            xs_t = big.tile([B, NCHUNK * B], BF16)   # stacked X (2.3 KB)
            # squash/softmax scratch aliased into tmp (live only while the
            # block scratch region is idle)
            tsq = tmp[:, 0:1024].bitcast(F32)        # [B, 512] f32
            smalls = tmp[:, 1024:1536].bitcast(F32)  # [B, 256] f32
            s2 = smalls[:, 0:N]
            a1 = smalls[:, N:2 * N]
            r1 = smalls[:, 2 * N:3 * N]
            rt = smalls[:, 3 * N:4 * N]
            ssum = tmp[:, 1536:1824].bitcast(F32)    # [B, 144] f32

            nc.vector.memset(eps_t[:], EPS)
            nc.vector.memset(bb[:], 0.0)

            # ---------- create: s0 via stacked matmuls ----------
            nc.sync.dma_start(out=xs_t[:], in_=xs_d[:, :, :])
            s0ps = ps0pool.tile([B, ND], F32)
            for c in range(NCHUNK):
                wst = stream.tile([B, ND], BF16, tag="wst")
                nc.sync.dma_start(out=wst[:], in_=ws_d[:, c, :])
                nc.tensor.matmul(
                    s0ps[:],
                    lhsT=_off(xs_t[:], c * B, [[1, B]]),
                    rhs=wst[:],
                    start=(c == 0),
                    stop=(c == NCHUNK - 1),
                )
            # s_acc = s0 / N  (uniform first-round coupling coefficients)
            nc.scalar.mul(out=s_acc[:], in_=s0ps[:], mul=1.0 / N)

            def allreduce_s():
                ar_in = dram.tile([B, ND], F32, tag="arin")
                ar_out = dram.tile([B, ND], F32, tag="arout")
                nc.gpsimd.dma_start(out=ar_in[:], in_=s_acc[:])
                nc.gpsimd.collective_compute(
                    "AllReduce",
                    mybir.AluOpType.add,
                    replica_groups=[list(range(NCORES))],
                    ins=[ar_in.opt()],
                    outs=[ar_out.opt()],
                )
                nc.gpsimd.dma_start(out=s_acc[:], in_=ar_out[:])

            allreduce_s()  # overlaps the hat stream below

            # ---------- create: hat via per-i matmuls ----------
            NCH = IL // CH
            for ic in range(NCH):
                wt = stream.tile([DIN, CH, ND], BF16, tag="wt")
                xch = stream.tile([DIN, CH, B], BF16, tag="xch")
                nc.sync.dma_start(out=wt[:], in_=w_d[:, ic * CH:(ic + 1) * CH, :])
                nc.gpsimd.dma_start(out=xch[:], in_=x_d[:, ic * CH:(ic + 1) * CH, :])
                ps = pspool.tile([B, CH * ND], F32)
                for j in range(CH):
                    nc.tensor.matmul(
                        ps[:, j * ND:(j + 1) * ND],
                        lhsT=xch[:, j, :],
                        rhs=wt[:, j, :],
                        start=True,
                        stop=True,
                    )
                dst = _off(hat[:], ic * CH * ND, [[1, CH * ND]])
                if ic % 2 == 0:
                    nc.vector.tensor_copy(dst, ps[:])
                else:
                    nc.scalar.copy(out=dst, in_=ps[:])

            # ---------- routing passes ----------
            def squash(last: bool):
                nc.vector.tensor_mul(tsq, s_acc[:], s_acc[:])
                nc.vector.reduce_sum(
                    out=s2, in_=_ap(tsq, [[D, N], [1, D]]),
                    axis=mybir.AxisListType.X)
                nc.scalar.add(out=a1, in_=s2, add=1.0)
                nc.vector.reciprocal(out=r1, in_=a1)
                nc.vector.tensor_mul(r1, r1, s2)          # s2/(1+s2)
                nc.scalar.activation(
                    out=rt, in_=s2,
                    func=mybir.ActivationFunctionType.Sqrt,
                    bias=eps_t[:], scale=1.0)
                nc.vector.reciprocal(out=rt, in_=rt)
                nc.vector.tensor_mul(r1, r1, rt)          # full scale [B,N]
                nc.vector.tensor_mul(
                    _ap(outv[:], [[D, N], [1, D]]),
                    _ap(s_acc[:], [[D, N], [1, D]]),
                    _ap(r1, [[1, N], [0, D]]))
                if not last:
                    nc.vector.tensor_copy(outbf[:], outv[:])

            def bb_block_dve(blk):
                i0 = blk * BI
                # tmp[0:4096] = hat_blk * out (broadcast over i)
                nc.vector.tensor_mul(
                    _ap(tmp[:], [[ND, BI], [1, ND]]),
                    _off(hat[:], i0 * ND, [[ND, BI], [1, ND]]),
                    _ap(outbf[:], [[0, BI], [1, ND]]))
                # d-tree: 16 -> 8 -> 4 -> 2 -> 1
                S = BI * N  # 256 segments
                nc.vector.tensor_add(
                    _off(tmp[:], 4096, [[8, S], [1, 8]]),
                    _ap(tmp[:], [[D, S], [1, 8]]),
                    _off(tmp[:], 8, [[D, S], [1, 8]]))
                nc.vector.tensor_add(
                    _ap(tmp[:], [[4, S], [1, 4]]),
                    _off(tmp[:], 4096, [[8, S], [1, 4]]),
                    _off(tmp[:], 4096 + 4, [[8, S], [1, 4]]))
                nc.vector.tensor_add(
                    _off(tmp[:], 1024, [[2, S], [1, 2]]),
                    _ap(tmp[:], [[4, S], [1, 2]]),
                    _off(tmp[:], 2, [[4, S], [1, 2]]))
                nc.vector.tensor_add(
                    _off(tmp[:], 1536, [[1, S]]),
                    _off(tmp[:], 1024, [[2, S]]),
                    _off(tmp[:], 1025, [[2, S]]))
                bbs = _off(bb[:], i0 * N, [[1, S]])
                nc.vector.tensor_add(bbs, bbs, _off(tmp[:], 1536, [[1, S]]))

            def bb_block_gp(gblk):
                i0 = DVE_I + gblk * BIG
                S = BIG * N  # 128 segments
                nc.gpsimd.tensor_mul(
                    _ap(tmpg[:], [[ND, BIG], [1, ND]]),
                    _off(hat[:], i0 * ND, [[ND, BIG], [1, ND]]),
                    _ap(outbf[:], [[0, BIG], [1, ND]]))
                nc.gpsimd.tensor_add(
                    _off(tmpg[:], 2048, [[8, S], [1, 8]]),
                    _ap(tmpg[:], [[D, S], [1, 8]]),
                    _off(tmpg[:], 8, [[D, S], [1, 8]]))
                nc.gpsimd.tensor_add(
                    _ap(tmpg[:], [[4, S], [1, 4]]),
                    _off(tmpg[:], 2048, [[8, S], [1, 4]]),
                    _off(tmpg[:], 2048 + 4, [[8, S], [1, 4]]))
                nc.gpsimd.tensor_add(
                    _off(tmpg[:], 512, [[2, S], [1, 2]]),
                    _ap(tmpg[:], [[4, S], [1, 2]]),
                    _off(tmpg[:], 2, [[4, S], [1, 2]]))
                nc.gpsimd.tensor_add(
                    _off(tmpg[:], 768, [[1, S]]),
                    _off(tmpg[:], 512, [[2, S]]),
                    _off(tmpg[:], 513, [[2, S]]))
                bbs = _off(bb[:], i0 * N, [[1, S]])
                nc.gpsimd.tensor_add(bbs, bbs, _off(tmpg[:], 768, [[1, S]]))

            def softmax():
                nc.scalar.activation(
                    out=ee[:], in_=bb[:],
                    func=mybir.ActivationFunctionType.Exp,
                    bias=eps_t[:], scale=1.0)
                nc.vector.reduce_sum(
                    out=ssum, in_=_ap(ee[:], [[N, IL], [1, N]]),
                    axis=mybir.AxisListType.X)
                nc.vector.reciprocal(out=ssum, in_=ssum)
                nc.vector.tensor_mul(
                    _ap(ee[:], [[N, IL], [1, N]]),
                    _ap(ee[:], [[N, IL], [1, N]]),
                    _ap(ssum, [[1, IL], [0, N]]))

            def s_block_dve(blk):
                i0 = blk * BI
                H = BI // 2
                for h in range(2):
                    cx = cexpp.tile([B, H * ND], BF16, tag="cexp")
                    nc.scalar.copy(
                        out=_ap(cx[:], [[ND, H], [D, N], [1, D]]),
                        in_=_off(ee[:], (i0 + h * H) * N,
                                 [[N, H], [1, N], [0, D]]))
                    nc.vector.tensor_mul(
                        _off(tmp[:], h * H * ND, [[ND, H], [1, ND]]),
                        _off(hat[:], (i0 + h * H) * ND, [[ND, H], [1, ND]]),
                        _ap(cx[:], [[ND, H], [1, ND]]))
                # i-tree: 8 -> 4 -> 2 -> 1, then accumulate into s_acc
                nc.vector.tensor_add(
                    _off(tmp[:], 4096, [[ND, 4], [1, ND]]),
                    _ap(tmp[:], [[ND, 4], [1, ND]]),
                    _off(tmp[:], 2048, [[ND, 4], [1, ND]]))
                nc.vector.tensor_add(
                    _ap(tmp[:], [[ND, 2], [1, ND]]),
                    _off(tmp[:], 4096, [[ND, 2], [1, ND]]),
                    _off(tmp[:], 5120, [[ND, 2], [1, ND]]))
                nc.vector.tensor_add(
                    _off(tmp[:], 1024, [[256, 2], [1, 256]]),
                    _ap(tmp[:], [[256, 2], [1, 256]]),
                    _off(tmp[:], ND, [[256, 2], [1, 256]]))
                nc.vector.tensor_add(
                    _ap(s_acc[:], [[256, 2], [1, 256]]),
                    _ap(s_acc[:], [[256, 2], [1, 256]]),
                    _off(tmp[:], 1024, [[256, 2], [1, 256]]))

            def s_block_gp(gblk):
                i0 = DVE_I + gblk * BIG
                # gpsimd reads the c broadcast directly (software engine --
                # strided operands cost the same); no cexp staging needed.
                nc.gpsimd.tensor_mul(
                    _ap(tmpg[:], [[ND, BIG], [D, N], [1, D]]),
                    _off(hat[:], i0 * ND, [[ND, BIG], [D, N], [1, D]]),
                    _off(ee[:], i0 * N, [[N, BIG], [1, N], [0, D]]))
                nc.gpsimd.tensor_add(
                    _off(tmpg[:], 2048, [[1, 1024]]),
                    _ap(tmpg[:], [[1, 1024]]),
                    _off(tmpg[:], 1024, [[1, 1024]]))
                nc.gpsimd.tensor_add(
                    _off(tmpg[:], 3072, [[1, ND]]),
                    _off(tmpg[:], 2048, [[1, ND]]),
                    _off(tmpg[:], 2048 + ND, [[1, ND]]))
                nc.gpsimd.tensor_add(
                    s_accg[:], s_accg[:], _off(tmpg[:], 3072, [[1, ND]]))

            for r in range(ROUTINGS):
                squash(last=(r == ROUTINGS - 1))
                if r == ROUTINGS - 1:
                    break
                # ---- bb += sum_d hat*out ----
                for gblk in range(NGBLK):
                    bb_block_gp(gblk)
                for blk in range(NBLK):
                    bb_block_dve(blk)
                softmax()
                # ---- s = sum_i c*hat ----
                nc.vector.memset(s_acc[:], 0.0)
                if GP_I:
                    nc.gpsimd.memset(s_accg[:], 0.0)
                for gblk in range(NGBLK):
                    s_block_gp(gblk)
                for blk in range(NBLK):
                    s_block_dve(blk)
                if GP_I:
                    nc.vector.tensor_add(s_acc[:], s_acc[:], s_accg[:])
                allreduce_s()

            nc.sync.dma_start(out=out_d[:], in_=outv[:])

    nc.compile()
    return nc


_NC_CACHE = None


def kernel(inputs: np.ndarray, W: np.ndarray) -> np.ndarray:
    global _NC_CACHE
    if _NC_CACHE is None:
        _NC_CACHE = build_nc()
    nc = _NC_CACHE

    inputs = np.ascontiguousarray(inputs, dtype=np.float32)
    W = np.ascontiguousarray(W, dtype=np.float32)
    # bf16 once, full tensors
    x_bf = inputs.astype(BF)                      # [B, I, DIN]
    w_bf = W.transpose(3, 1, 0, 2).reshape(DIN, I, ND).astype(BF)  # [k, i, nd]

    in_maps = []
    for c in range(NCORES):
        sl = slice(c * IL, (c + 1) * IL)
        x_c = np.ascontiguousarray(x_bf[:, sl, :].transpose(2, 1, 0))  # [k,i,b]
        w_c = np.ascontiguousarray(w_bf[:, sl, :])                     # [k,i,nd]
        # stacked layouts for the s0 chain: p = (i%16)*8 + k
        xs_c = np.ascontiguousarray(
            x_bf[:, sl, :].reshape(B, NCHUNK, 16, DIN)
            .transpose(2, 3, 1, 0).reshape(128, NCHUNK, B))
        ws_c = np.ascontiguousarray(
            w_c.reshape(DIN, NCHUNK, 16, ND)
            .transpose(2, 0, 1, 3).reshape(128, NCHUNK, ND))
        in_maps.append({"x": x_c, "w": w_c, "xs": xs_c, "ws": ws_c})

    trace = bool(int(os.environ.get("CAPS_TRACE", "0")))
    res = bass_utils.run_bass_kernel_spmd(
        nc, in_maps, core_ids=list(range(NCORES)), trace=trace)
    if trace and res.exec_time_ns is not None:
        print(f"HW exec time: {res.exec_time_ns} ns")
    return res.results[0]["out"].reshape(B, N, D).astype(np.float32)


# revision 8
# speedup vs baseline: 1.2095x; 1.2095x over previous
"""CapsuleLayer dynamic-routing kernel for 8 Trainium2 NeuronCores.

I-sharding: each core owns 144 of the 1152 input capsules.

Create phase (all bf16 on the PE):
  - s0 = sum_i hat_i via 9 stacked matmuls: lhsT = xs[(i16,k8)=128, b],
    rhs = ws[(i16,k8)=128, nd] accumulated in one PSUM chain.
  - hat_i = X_i^T W_i per i: 144 serial bf16 matmuls [8,128]x[8,512],
    PSUM drained to SBUF hat (bf16) by copies alternating DVE/ScalarE.
  - AllReduce(s0) overlaps the hat matmul/copy stream.

Routing (b on partitions, free-dim elementwise on DVE at 2x bf16):
  - reduces done as binary add-trees (TT 2x) instead of 1x tensor_reduce.
  - c-broadcast over d materialized by ScalarE (cexp) in half-blocks so
    the DVE multiply gets contiguous bf16 operands.
  - optional gpsimd side-lane takes the last GP_I capsules per pass.
  - Cross-core: AllReduce of s_partial [128,512] fp32, 3x.
Every core computes the identical final output; core 0's is returned.
"""

import os
import numpy as np
import ml_dtypes

import concourse.bass as bass
import concourse.bacc as bacc
import concourse.tile as tile
import concourse.mybir as mybir
from concourse import bass_utils

B, I, DIN = 128, 1152, 8
N, D = 32, 16
ND = N * D  # 512
NCORES = 8
IL = I // NCORES  # 144
EPS = 1e-7
ROUTINGS = 3
F32 = mybir.dt.float32
BF16 = mybir.dt.bfloat16

CH = 3        # i-chunk for X/W streaming in create (48 chunks)
BI = 8        # i-block for DVE routing passes
GP_I = 0      # trailing i's handled by gpsimd (multiple of 4; 0 disables)
BIG = 4       # gpsimd i-block
NCHUNK = 9    # stacked s0 chunks (16 i's each)

BF = np.dtype(ml_dtypes.bfloat16)


def _ap(ap: bass.AP, dims) -> bass.AP:
    """Rebuild `ap` with explicit free [step,count] dims (partition dim kept)."""
    return bass.AP(tensor=ap.tensor, offset=ap.offset, ap=[ap.ap[0]] + list(dims))


def _off(ap: bass.AP, elems: int, dims) -> bass.AP:
    """Like _ap but with an extra offset in elements of the tensor dtype."""
    return bass.AP(
        tensor=ap.tensor,
        offset=ap.offset + elems,
        ap=[ap.ap[0]] + list(dims),
    )


def build_nc():
    nc = bacc.Bacc(
        "TRN2",
        target_bir_lowering=False,
        debug=False,
        enable_asserts=True,
        num_devices=NCORES,
    )
    x_d = nc.dram_tensor("x", [DIN, IL, B], BF16, kind="ExternalInput").ap()
    w_d = nc.dram_tensor("w", [DIN, IL, ND], BF16, kind="ExternalInput").ap()
    xs_d = nc.dram_tensor("xs", [128, NCHUNK, B], BF16, kind="ExternalInput").ap()
    ws_d = nc.dram_tensor("ws", [128, NCHUNK, ND], BF16, kind="ExternalInput").ap()
    out_d = nc.dram_tensor("out", [B, ND], F32, kind="ExternalOutput").ap()

    DVE_I = IL - GP_I          # i's handled by DVE blocks
    NBLK = DVE_I // BI
    NGBLK = GP_I // BIG

    with tile.TileContext(nc) as tc:
        with (
            tc.tile_pool(name="big", bufs=1) as big,
            tc.tile_pool(name="stream", bufs=2) as stream,
            tc.tile_pool(name="cexpp", bufs=2) as cexpp,
            tc.tile_pool(name="ps", bufs=2, space="PSUM") as pspool,
            tc.tile_pool(name="ps0", bufs=1, space="PSUM") as ps0pool,
            tc.tile_pool(name="dram", bufs=1, space="DRAM") as dram,
        ):
            hat = big.tile([B, IL * ND], BF16)       # 147.5 KB/part
            bb = big.tile([B, IL * N], BF16)         # 9.2 KB
            ee = big.tile([B, IL * N], BF16)         # 9.2 KB
            s_acc = big.tile([B, ND], F32)
            if GP_I:
                s_accg = big.tile([B, ND], F32)
            outv = big.tile([B, ND], F32)
            outbf = big.tile([B, ND], BF16)
            eps_t = big.tile([B, 1], F32)
            tmp = big.tile([B, 12 * ND], BF16)       # 12 KB DVE scratch
            if GP_I:
                tmpg = big.tile([B, 3584], BF16)     # 7 KB gpsimd scratch
            xs_t = big.tile([B, NCHUNK * B], BF16)   # stacked X (2.3 KB)
            # squash/softmax scratch aliased into tmp (live only while the
            # block scratch region is idle)
            tsq = tmp[:, 0:1024].bitcast(F32)        # [B, 512] f32
            smalls = tmp[:, 1024:1536].bitcast(F32)  # [B, 256] f32
            s2 = smalls[:, 0:N]
            a1 = smalls[:, N:2 * N]
            r1 = smalls[:, 2 * N:3 * N]
            rt = smalls[:, 3 * N:4 * N]
            ssum = tmp[:, 1536:1824].bitcast(F32)    # [B, 144] f32

            nc.vector.memset(eps_t[:], EPS)
            nc.vector.memset(bb[:], 0.0)

            # ---------- create: s0 via stacked matmuls ----------
            nc.sync.dma_start(out=xs_t[:], in_=xs_d[:, :, :])
            s0ps = ps0pool.tile([B, ND], F32)
            for c in range(NCHUNK):
                wst = stream.tile([B, ND], BF16, tag="wst")
                nc.sync.dma_start(out=wst[:], in_=ws_d[:, c, :])
                nc.tensor.matmul(
                    s0ps[:],
                    lhsT=_off(xs_t[:], c * B, [[1, B]]),
                    rhs=wst[:],
                    start=(c == 0),
                    stop=(c == NCHUNK - 1),
                )
            # s_acc = s0 / N  (uniform first-round coupling coefficients)
            nc.scalar.mul(out=s_acc[:], in_=s0ps[:], mul=1.0 / N)

            def allreduce_s():
                ar_in = dram.tile([B, ND], F32, tag="arin")
                ar_out = dram.tile([B, ND], F32, tag="arout")
                nc.gpsimd.dma_start(out=ar_in[:], in_=s_acc[:])
                nc.gpsimd.collective_compute(
                    "AllReduce",
                    mybir.AluOpType.add,
                    replica_groups=[list(range(NCORES))],
                    ins=[ar_in.opt()],
                    outs=[ar_out.opt()],
                )
                nc.gpsimd.dma_start(out=s_acc[:], in_=ar_out[:])

            allreduce_s()  # overlaps the hat stream below

            # ---------- create: hat via per-i matmuls ----------
            NCH = IL // CH
            for ic in range(NCH):
                wt = stream.tile([DIN, CH, ND], BF16, tag="wt")
                xch = stream.tile([DIN, CH, B], BF16, tag="xch")
                nc.sync.dma_start(out=wt[:], in_=w_d[:, ic * CH:(ic + 1) * CH, :])
                nc.gpsimd.dma_start(out=xch[:], in_=x_d[:, ic * CH:(ic + 1) * CH, :])
                ps = pspool.tile([B, CH * ND], F32)
                for j in range(CH):
                    nc.tensor.matmul(
                        ps[:, j * ND:(j + 1) * ND],
                        lhsT=xch[:, j, :],
                        rhs=wt[:, j, :],
                        start=True,
                        stop=True,
                    )
                dst = _off(hat[:], ic * CH * ND, [[ND, CH], [1, ND]])
                nc.scalar.copy(out=dst, in_=_ap(ps[:], [[ND, CH], [1, ND]]))

            # ---------- routing passes ----------
            def squash(last: bool):
                nc.vector.tensor_mul(tsq, s_acc[:], s_acc[:])
                nc.vector.reduce_sum(
                    out=s2, in_=_ap(tsq, [[D, N], [1, D]]),
                    axis=mybir.AxisListType.X)
                nc.scalar.add(out=a1, in_=s2, add=1.0)
                nc.vector.reciprocal(out=r1, in_=a1)
                nc.vector.tensor_mul(r1, r1, s2)          # s2/(1+s2)
                nc.scalar.activation(
                    out=rt, in_=s2,
                    func=mybir.ActivationFunctionType.Sqrt,
                    bias=eps_t[:], scale=1.0)
                nc.vector.reciprocal(out=rt, in_=rt)
                nc.vector.tensor_mul(r1, r1, rt)          # full scale [B,N]
                nc.vector.tensor_mul(
                    _ap(outv[:], [[D, N], [1, D]]),
                    _ap(s_acc[:], [[D, N], [1, D]]),
                    _ap(r1, [[1, N], [0, D]]))
                if not last:
                    nc.vector.tensor_copy(outbf[:], outv[:])

            def bb_block_dve(blk):
                i0 = blk * BI
                # tmp[0:4096] = hat_blk * out (broadcast over i)
                nc.vector.tensor_mul(
                    _ap(tmp[:], [[ND, BI], [1, ND]]),
                    _off(hat[:], i0 * ND, [[ND, BI], [1, ND]]),
                    _ap(outbf[:], [[0, BI], [1, ND]]))
                # d-tree: 16 -> 8 -> 4 -> 2 -> 1
                S = BI * N  # 256 segments
                nc.vector.tensor_add(
                    _off(tmp[:], 4096, [[8, S], [1, 8]]),
                    _ap(tmp[:], [[D, S], [1, 8]]),
                    _off(tmp[:], 8, [[D, S], [1, 8]]))
                nc.vector.tensor_add(
                    _ap(tmp[:], [[4, S], [1, 4]]),
                    _off(tmp[:], 4096, [[8, S], [1, 4]]),
                    _off(tmp[:], 4096 + 4, [[8, S], [1, 4]]))
                nc.vector.tensor_add(
                    _off(tmp[:], 1024, [[2, S], [1, 2]]),
                    _ap(tmp[:], [[4, S], [1, 2]]),
                    _off(tmp[:], 2, [[4, S], [1, 2]]))
                nc.vector.tensor_add(
                    _off(tmp[:], 1536, [[1, S]]),
                    _off(tmp[:], 1024, [[2, S]]),
                    _off(tmp[:], 1025, [[2, S]]))
                bbs = _off(bb[:], i0 * N, [[1, S]])
                nc.vector.tensor_add(bbs, bbs, _off(tmp[:], 1536, [[1, S]]))

            def bb_block_gp(gblk):
                i0 = DVE_I + gblk * BIG
                S = BIG * N  # 128 segments
                nc.gpsimd.tensor_mul(
                    _ap(tmpg[:], [[ND, BIG], [1, ND]]),
                    _off(hat[:], i0 * ND, [[ND, BIG], [1, ND]]),
                    _ap(outbf[:], [[0, BIG], [1, ND]]))
                nc.gpsimd.tensor_add(
                    _off(tmpg[:], 2048, [[8, S], [1, 8]]),
                    _ap(tmpg[:], [[D, S], [1, 8]]),
                    _off(tmpg[:], 8, [[D, S], [1, 8]]))
                nc.gpsimd.tensor_add(
                    _ap(tmpg[:], [[4, S], [1, 4]]),
                    _off(tmpg[:], 2048, [[8, S], [1, 4]]),
                    _off(tmpg[:], 2048 + 4, [[8, S], [1, 4]]))
                nc.gpsimd.tensor_add(
                    _off(tmpg[:], 512, [[2, S], [1, 2]]),
                    _ap(tmpg[:], [[4, S], [1, 2]]),
                    _off(tmpg[:], 2, [[4, S], [1, 2]]))
                nc.gpsimd.tensor_add(
                    _off(tmpg[:], 768, [[1, S]]),
                    _off(tmpg[:], 512, [[2, S]]),
                    _off(tmpg[:], 513, [[2, S]]))
                bbs = _off(bb[:], i0 * N, [[1, S]])
                nc.gpsimd.tensor_add(bbs, bbs, _off(tmpg[:], 768, [[1, S]]))

            def softmax():
                nc.scalar.activation(
                    out=ee[:], in_=bb[:],
                    func=mybir.ActivationFunctionType.Exp,
                    bias=eps_t[:], scale=1.0)
                nc.vector.reduce_sum(
                    out=ssum, in_=_ap(ee[:], [[N, IL], [1, N]]),
                    axis=mybir.AxisListType.X)
                nc.vector.reciprocal(out=ssum, in_=ssum)
                nc.vector.tensor_mul(
                    _ap(ee[:], [[N, IL], [1, N]]),
                    _ap(ee[:], [[N, IL], [1, N]]),
                    _ap(ssum, [[1, IL], [0, N]]))

            def s_block_dve(blk):
                i0 = blk * BI
                cx = cexpp.tile([B, BI * ND], BF16, tag="cexp")
                nc.scalar.copy(
                    out=_ap(cx[:], [[ND, BI], [D, N], [1, D]]),
                    in_=_off(ee[:], i0 * N, [[N, BI], [1, N], [0, D]]))
                nc.vector.tensor_mul(
                    _ap(tmp[:], [[ND, BI], [1, ND]]),
                    _off(hat[:], i0 * ND, [[ND, BI], [1, ND]]),
                    _ap(cx[:], [[ND, BI], [1, ND]]))
                # i-tree: 8 -> 4 -> 2 -> 1, then accumulate into s_acc
                nc.vector.tensor_add(
                    _off(tmp[:], 4096, [[ND, 4], [1, ND]]),
                    _ap(tmp[:], [[ND, 4], [1, ND]]),
                    _off(tmp[:], 2048, [[ND, 4], [1, ND]]))
                nc.vector.tensor_add(
                    _ap(tmp[:], [[ND, 2], [1, ND]]),
                    _off(tmp[:], 4096, [[ND, 2], [1, ND]]),
                    _off(tmp[:], 5120, [[ND, 2], [1, ND]]))
                nc.vector.tensor_add(
                    _off(tmp[:], 1024, [[256, 2], [1, 256]]),
                    _ap(tmp[:], [[256, 2], [1, 256]]),
                    _off(tmp[:], ND, [[256, 2], [1, 256]]))
                nc.vector.tensor_add(
                    _ap(s_acc[:], [[256, 2], [1, 256]]),
                    _ap(s_acc[:], [[256, 2], [1, 256]]),
                    _off(tmp[:], 1024, [[256, 2], [1, 256]]))

            def s_block_gp(gblk):
                i0 = DVE_I + gblk * BIG
                # gpsimd reads the c broadcast directly (software engine --
                # strided operands cost the same); no cexp staging needed.
                nc.gpsimd.tensor_mul(
                    _ap(tmpg[:], [[ND, BIG], [D, N], [1, D]]),
                    _off(hat[:], i0 * ND, [[ND, BIG], [D, N], [1, D]]),
                    _off(ee[:], i0 * N, [[N, BIG], [1, N], [0, D]]))
                nc.gpsimd.tensor_add(
                    _off(tmpg[:], 2048, [[1, 1024]]),
                    _ap(tmpg[:], [[1, 1024]]),
                    _off(tmpg[:], 1024, [[1, 1024]]))
                nc.gpsimd.tensor_add(
                    _off(tmpg[:], 3072, [[1, ND]]),
                    _off(tmpg[:], 2048, [[1, ND]]),
                    _off(tmpg[:], 2048 + ND, [[1, ND]]))
                nc.gpsimd.tensor_add(
                    s_accg[:], s_accg[:], _off(tmpg[:], 3072, [[1, ND]]))

            for r in range(ROUTINGS):
                squash(last=(r == ROUTINGS - 1))
                if r == ROUTINGS - 1:
                    break
                # ---- bb += sum_d hat*out ----
                for gblk in range(NGBLK):
                    bb_block_gp(gblk)
                for blk in range(NBLK):
                    bb_block_dve(blk)
                softmax()
                # ---- s = sum_i c*hat ----
                nc.vector.memset(s_acc[:], 0.0)
                if GP_I:
                    nc.gpsimd.memset(s_accg[:], 0.0)
                for gblk in range(NGBLK):
                    s_block_gp(gblk)
                for blk in range(NBLK):
                    s_block_dve(blk)
                if GP_I:
                    nc.vector.tensor_add(s_acc[:], s_acc[:], s_accg[:])
                allreduce_s()

            nc.sync.dma_start(out=out_d[:], in_=outv[:])

    nc.compile()
    return nc


_NC_CACHE = None


def kernel(inputs: np.ndarray, W: np.ndarray) -> np.ndarray:
    global _NC_CACHE
    if _NC_CACHE is None:
        _NC_CACHE = build_nc()
    nc = _NC_CACHE

    inputs = np.ascontiguousarray(inputs, dtype=np.float32)
    W = np.ascontiguousarray(W, dtype=np.float32)
    # bf16 once, full tensors
    x_bf = inputs.astype(BF)                      # [B, I, DIN]
    w_bf = W.transpose(3, 1, 0, 2).reshape(DIN, I, ND).astype(BF)  # [k, i, nd]

    in_maps = []
    for c in range(NCORES):
        sl = slice(c * IL, (c + 1) * IL)
        x_c = np.ascontiguousarray(x_bf[:, sl, :].transpose(2, 1, 0))  # [k,i,b]
        w_c = np.ascontiguousarray(w_bf[:, sl, :])                     # [k,i,nd]
        # stacked layouts for the s0 chain: p = (i%16)*8 + k
        xs_c = np.ascontiguousarray(
            x_bf[:, sl, :].reshape(B, NCHUNK, 16, DIN)
            .transpose(2, 3, 1, 0).reshape(128, NCHUNK, B))
        ws_c = np.ascontiguousarray(
            w_c.reshape(DIN, NCHUNK, 16, ND)
            .transpose(2, 0, 1, 3).reshape(128, NCHUNK, ND))
        in_maps.append({"x": x_c, "w": w_c, "xs": xs_c, "ws": ws_c})

    trace = bool(int(os.environ.get("CAPS_TRACE", "0")))
    res = bass_utils.run_bass_kernel_spmd(
        nc, in_maps, core_ids=list(range(NCORES)), trace=trace)
    if trace and res.exec_time_ns is not None:
        print(f"HW exec time: {res.exec_time_ns} ns")
    return res.results[0]["out"].reshape(B, N, D).astype(np.float32)
